# revision 16
# baseline (speedup 1.0000x reference)
"""MambaVisionBlock Trainium2 Bass kernel.

Sharding: data-parallel over batch B=8 across 8 NeuronCores (1 batch/core),
all parameters replicated.  Per-core problem: x [4096, 256].

Layout strategy: feature-major activations [d (2x128 partitions), t (free)],
processed in 8 chunks of T=512 tokens.
 - x is DMA'd token-major and PE-transposed to feature-major (8 transposes per
   chunk); the final output is PE-transposed back.
 - All weight matrices are PE-transposed once at setup into [K-part, M-free]
   lhsT layout for the tensor engine.
 - Matmuls run in float32r (1 cycle/row at N=512; plain fp32 is 4 cycles/row).
 - LayerNorm stats: sum(x), sum(x^2) via ones-matmul on the PE; the ones lhsT
   has M=128 so the sums come out already broadcast across partitions.
 - Causal depthwise conv(k=3): shifted-AP fused multiply-adds on DVE with a
   2-element halo carried between chunks.
 - cumsum along L: DVE tensor_tensor_scan, chunk-chained via initial=prev[-1:].
"""

import sys

if "/opt/trn_rl_repo" not in sys.path:
    sys.path.insert(0, "/opt/trn_rl_repo")

import numpy as np

B, L, D = 8, 4096, 256
Dff = 1024
T = 512            # token chunk
NCH = L // T       # 8 chunks
NCORES = 8
LN_EPS = 1e-5

_CACHE = {}

WEIGHT_NAMES = [
    "ln1_w", "ln1_b", "in_proj_w", "conv_w", "conv_b", "ssm_B", "ssm_C",
    "ssm_D", "out_proj_w", "ln2_w", "ln2_b", "fc1_w", "fc1_b", "fc2_w",
    "fc2_b",
]


def _build(sim_compat=False):
    import concourse.tile as tile
    from concourse import bacc, mybir
    from concourse.masks import make_identity

    f32 = mybir.dt.float32
    f32r = mybir.dt.float32r
    ALU = mybir.AluOpType
    ACT = mybir.ActivationFunctionType

    nc = bacc.Bacc(trn_type="TRN2")

    # ---- DRAM I/O ----
    x_h = nc.dram_tensor("x", [L, D], f32, kind="ExternalInput")
    w_h = {}
    shapes = {
        "ln1_w": [D], "ln1_b": [D], "in_proj_w": [2 * D, D],
        "conv_w": [D, 1, 3], "conv_b": [D], "ssm_B": [D, 8], "ssm_C": [D, 8],
        "ssm_D": [D], "out_proj_w": [D, D], "ln2_w": [D], "ln2_b": [D],
        "fc1_w": [Dff, D], "fc1_b": [Dff], "fc2_w": [D, Dff], "fc2_b": [D],
    }
    for n in WEIGHT_NAMES:
        w_h[n] = nc.dram_tensor(n, shapes[n], f32, kind="ExternalInput")
    out_h = nc.dram_tensor("out", [L, D], f32, kind="ExternalOutput")

    x_ap = x_h[:, :]
    out_ap = out_h[:, :]

    def r(ap):
        return ap.bitcast(f32r)

    from contextlib import ExitStack
    with tile.TileContext(nc) as tc, ExitStack() as stack:
        pool_w = stack.enter_context(tc.tile_pool(name="weights", bufs=1))
        pool_a = stack.enter_context(tc.tile_pool(name="acts", bufs=2))
        pool_g = stack.enter_context(tc.tile_pool(name="gelu", bufs=2))
        pool_s = stack.enter_context(tc.tile_pool(name="stats", bufs=2))
        psA = stack.enter_context(tc.tile_pool(name="psA", bufs=2, space="PSUM"))
        psB = stack.enter_context(tc.tile_pool(name="psB", bufs=1, space="PSUM"))
        psC = stack.enter_context(tc.tile_pool(name="psC", bufs=4, space="PSUM"))

        # ---- constants ----
        ident = pool_w.tile([128, 128], f32, tag="ident")
        make_identity(nc, ident)
        ones_f = pool_w.tile([128, 128], f32, tag="ones_f")
        nc.vector.memset(ones_f, 1.0)
        ones128 = pool_w.tile([128, 128], f32r, tag="ones")
        nc.vector.tensor_copy(ones128[:], ones_f)
        zerosT = pool_w.tile([128, T], f32, tag="zeros")
        nc.vector.memset(zerosT, 0.0)
        epsT = pool_w.tile([128, 1], f32, tag="eps")
        nc.vector.memset(epsT, LN_EPS)

        # ---- per-feature vectors -> [128, nblk] (partition = d % 128, blk = d // 128) ----
        def vec_tile(name, nblk):
            t_ = pool_w.tile([128, nblk], f32, tag="v_" + name)
            nc.sync.dma_start(out=t_, in_=w_h[name][:].rearrange("(b p) -> p b", p=128))
            return t_

        ln1w = vec_tile("ln1_w", 2)
        ln1b = vec_tile("ln1_b", 2)
        ln2w = vec_tile("ln2_w", 2)
        ln2b = vec_tile("ln2_b", 2)
        convb = vec_tile("conv_b", 2)
        ssmD = vec_tile("ssm_D", 2)
        fc1b = vec_tile("fc1_b", 8)
        fc2b = vec_tile("fc2_b", 2)

        cw = pool_w.tile([128, 2, 3], f32, tag="convw")
        nc.sync.dma_start(out=cw, in_=w_h["conv_w"][:, 0, :].rearrange("(b p) k -> p b k", p=128))

        ssmB = pool_w.tile([128, 2, 8], f32, tag="ssmB")
        nc.sync.dma_start(out=ssmB, in_=w_h["ssm_B"][:].rearrange("(b p) s -> p b s", p=128))
        ssmC = pool_w.tile([128, 2, 8], f32, tag="ssmC")
        nc.sync.dma_start(out=ssmC, in_=w_h["ssm_C"][:].rearrange("(b p) s -> p b s", p=128))
        bcprod = pool_w.tile([128, 2, 8], f32, tag="bcprod")
        nc.vector.tensor_mul(bcprod, ssmB, ssmC)
        bc = pool_w.tile([128, 2], f32, tag="bc")
        nc.vector.tensor_reduce(bc, bcprod, axis=mybir.AxisListType.X, op=ALU.add)

        # ---- weight transposes: W [E, D] -> lhsT [d-part, e-free] ----
        # w_inT [128, db, 512], w_outT [128, db, 256], w1T [128, db, 1024],
        # w2T [128, fb, 256]
        w_inT = pool_w.tile([128, 2, 512], f32, tag="w_inT")
        w_outT = pool_w.tile([128, 2, 256], f32, tag="w_outT")
        w1T = pool_w.tile([128, 2, 1024], f32, tag="w1T")
        w2T = pool_w.tile([128, 8, 256], f32, tag="w2T")

        # in_proj [512, 256] staged as [128, 4(eb), 256]
        st_in = pool_w.tile([128, 4, 256], f32, tag="wst_a")
        nc.sync.dma_start(out=st_in, in_=w_h["in_proj_w"][:].rearrange("(e p) d -> p e d", p=128))
        for db in range(2):
            ps = psA.tile([128, 512], f32, tag="ptr")
            for eb in range(4):
                nc.tensor.transpose(ps[:, eb * 128:(eb + 1) * 128],
                                    st_in[:, eb, db * 128:(db + 1) * 128], ident)
            nc.vector.tensor_copy(r(w_inT[:, db, :]), ps)

        # out_proj [256, 256] staged as [128, 2(ob), 256]
        st_out = pool_w.tile([128, 2, 256], f32, tag="wst_b")
        nc.sync.dma_start(out=st_out, in_=w_h["out_proj_w"][:].rearrange("(e p) d -> p e d", p=128))
        for db in range(2):
            ps = psA.tile([128, 512], f32, tag="ptr")
            for ob in range(2):
                nc.tensor.transpose(ps[:, ob * 128:(ob + 1) * 128],
                                    st_out[:, ob, db * 128:(db + 1) * 128], ident)
            nc.vector.tensor_copy(r(w_outT[:, db, :]), ps[:, 0:256])

        # fc1 [1024, 256] staged as [128, 8(fb), 256]
        st_f1 = pool_w.tile([128, 8, 256], f32, tag="wst_c")
        nc.sync.dma_start(out=st_f1, in_=w_h["fc1_w"][:].rearrange("(e p) d -> p e d", p=128))
        for db in range(2):
            for half in range(2):
                ps = psA.tile([128, 512], f32, tag="ptr")
                for i in range(4):
                    fb = half * 4 + i
                    nc.tensor.transpose(ps[:, i * 128:(i + 1) * 128],
                                        st_f1[:, fb, db * 128:(db + 1) * 128], ident)
                nc.vector.tensor_copy(r(w1T[:, db, half * 512:(half + 1) * 512]), ps)

        # fc2 [256, 1024] staged as [128, 2(ob), 1024]
        st_f2 = pool_w.tile([128, 2, 1024], f32, tag="wst_d")
        nc.sync.dma_start(out=st_f2, in_=w_h["fc2_w"][:].rearrange("(e p) f -> p e f", p=128))
        for ob in range(2):
            for half in range(2):
                ps = psA.tile([128, 512], f32, tag="ptr")
                for i in range(4):
                    fb = half * 4 + i
                    nc.tensor.transpose(ps[:, i * 128:(i + 1) * 128],
                                        st_f2[:, ob, fb * 128:(fb + 1) * 128], ident)
                # ps = [128(f-part), 4(fb), 128(dout)] -> w2T[:, fb, ob*128: ]
                nc.vector.tensor_copy(
                    r(w2T[:, half * 4:(half + 1) * 4, ob * 128:(ob + 1) * 128]),
                    ps.rearrange("p (a b) -> p a b", a=4))

        # ---- silu / gelu emission (sim_compat: CoreSim lacks Silu/Gelu tables) ----
        def act_silu(out, in_, bias=0.0):
            if not sim_compat:
                nc.scalar.activation(out, in_, ACT.Silu, bias=bias)
            else:
                v = pool_s.tile(list(out.shape), f32, tag="simv")
                nc.scalar.activation(v, in_, ACT.Identity, bias=bias)
                sg = pool_s.tile(list(out.shape), f32, tag="simsg")
                nc.scalar.activation(sg, v, ACT.Sigmoid)
                nc.vector.tensor_mul(out, v, sg)

        def act_gelu(out, in_, bias=0.0):
            if not sim_compat:
                nc.scalar.activation(out, in_, ACT.Gelu, bias=bias)
            else:
                v = pool_s.tile(list(out.shape), f32, tag="simv")
                nc.scalar.activation(v, in_, ACT.Identity, bias=bias)
                sg = pool_s.tile(list(out.shape), f32, tag="simsg")
                nc.scalar.activation(sg, v, ACT.Sigmoid, scale=1.702)
                nc.vector.tensor_mul(out, v, sg)

        # ---- LayerNorm (feature-major PE-stats path) ----
        def layer_norm(src, wv, bv, tag):
            # src [128, 2, T] -> returns h [128, 2, T]
            sq = pool_s.tile([128, 2, T], f32, tag="sq")
            for db in range(2):
                nc.vector.tensor_mul(r(sq[:, db]), src[:, db], src[:, db])
            pst = psB.tile([128, 2, T], f32, tag="st")
            for db in range(2):
                nc.tensor.matmul(pst[:, 0], ones128[:], r(src[:, db]),
                                 start=(db == 0), stop=(db == 1))
            for db in range(2):
                nc.tensor.matmul(pst[:, 1], ones128[:], r(sq[:, db]),
                                 start=(db == 0), stop=(db == 1))
            mun = pool_s.tile([128, T], f32, tag="mun")
            nc.scalar.mul(mun, pst[:, 0], -1.0 / D)
            mu2 = pool_s.tile([128, T], f32, tag="mu2")
            nc.vector.tensor_mul(mu2, mun, mun)
            sd = pool_s.tile([128, T], f32, tag="sd")
            nc.vector.scalar_tensor_tensor(
                out=sd, in0=pst[:, 1], scalar=1.0 / D, in1=mu2,
                op0=ALU.mult, op1=ALU.subtract)
            nc.scalar.activation(sd, sd, ACT.Sqrt, bias=epsT[:])
            rstd = pool_s.tile([128, T], f32, tag="rstd")
            nc.vector.reciprocal(rstd, sd)
            h = pool_a.tile([128, 2, T], f32, tag="h" + tag)
            for db in range(2):
                t0 = pool_s.tile([128, T], f32, tag="t0")
                nc.vector.tensor_add(t0, src[:, db], mun)
                nc.vector.scalar_tensor_tensor(
                    out=t0, in0=t0, scalar=wv[:, db:db + 1], in1=rstd,
                    op0=ALU.mult, op1=ALU.mult)
                nc.scalar.activation(r(h[:, db]), t0, ACT.Identity,
                                     bias=bv[:, db:db + 1])
            return h

        # ---- main chunk loop ----
        prev_xc = None
        prev_cum = None
        for c in range(NCH):
            tok = slice(c * T, (c + 1) * T)
            # 1. load x chunk token-major [128, 4, 256]
            x_tm = pool_a.tile([128, 4, 256], f32, tag="x_tm")
            nc.sync.dma_start(out=x_tm, in_=x_ap[tok, :].rearrange("(s p) d -> p s d", p=128))
            # 2. transpose -> feature-major xf [128, 2, T]
            xf = pool_a.tile([128, 2, T], f32, tag="xf")
            for db in range(2):
                ps = psA.tile([128, 512], f32, tag="ptr")
                for s in range(4):
                    nc.tensor.transpose(ps[:, s * 128:(s + 1) * 128],
                                        x_tm[:, s, db * 128:(db + 1) * 128], ident)
                nc.vector.tensor_copy(r(xf[:, db]), ps)
            # 3. LN1
            h = layer_norm(xf, ln1w, ln1b, "1")
            # 4. in_proj -> 4 psum tiles [128, T]
            pxz = []
            for eb in range(4):
                p = psC.tile([128, T], f32, tag="mm")
                for db in range(2):
                    nc.tensor.matmul(p, r(w_inT[:, db, eb * 128:(eb + 1) * 128]),
                                     r(h[:, db]), start=(db == 0), stop=(db == 1))
                pxz.append(p)
            # 5. xc: copy with 2-halo, conv3, +bias, silu
            xc = pool_a.tile([128, 2, T + 2], f32, tag="xc")
            for eb in range(2):
                nc.vector.tensor_copy(xc[:, eb, 2:], pxz[eb])
                if c == 0:
                    nc.vector.memset(xc[:, eb, 0:2], 0.0)
                else:
                    nc.vector.tensor_copy(xc[:, eb, 0:2], prev_xc[:, eb, T:T + 2])
            xcv = pool_a.tile([128, 2, T], f32, tag="xcv")
            for eb in range(2):
                tA = pool_s.tile([128, T], f32, tag="tA")
                nc.vector.tensor_scalar_mul(tA, xc[:, eb, 0:T], cw[:, eb, 0:1])
                nc.vector.scalar_tensor_tensor(
                    out=tA, in0=xc[:, eb, 1:T + 1], scalar=cw[:, eb, 1:2],
                    in1=tA, op0=ALU.mult, op1=ALU.add)
                nc.vector.scalar_tensor_tensor(
                    out=tA, in0=xc[:, eb, 2:T + 2], scalar=cw[:, eb, 2:3],
                    in1=tA, op0=ALU.mult, op1=ALU.add)
                act_silu(xcv[:, eb], tA, bias=convb[:, eb:eb + 1])
            # 6. z = silu(xz[:, 2:4])
            zt = pool_a.tile([128, 2, T], f32, tag="zt")
            for i in range(2):
                act_silu(zt[:, i], pxz[2 + i])
            # 7. cumsum along tokens (chunk-chained)
            cum = pool_a.tile([128, 2, T], f32, tag="cum")
            for eb in range(2):
                init = 0.0 if c == 0 else prev_cum[:, eb, T - 1:T]
                nc.vector.tensor_tensor_scan(
                    out=cum[:, eb], data0=xcv[:, eb], data1=zerosT,
                    initial=init, op0=ALU.add, op1=ALU.add)
            # 8. yz = (cum*bc + xcv*ssmD) * z
            t1 = pool_a.tile([128, 2, T], f32, tag="t1")
            t2 = pool_a.tile([128, 2, T], f32, tag="t2")
            for eb in range(2):
                nc.vector.scalar_tensor_tensor(
                    out=r(t1[:, eb]), in0=xcv[:, eb], scalar=ssmD[:, eb:eb + 1],
                    in1=zt[:, eb], op0=ALU.mult, op1=ALU.mult)
                nc.vector.scalar_tensor_tensor(
                    out=t2[:, eb], in0=cum[:, eb], scalar=bc[:, eb:eb + 1],
                    in1=zt[:, eb], op0=ALU.mult, op1=ALU.mult)
                nc.vector.tensor_add(r(t1[:, eb]), t1[:, eb], t2[:, eb])
            # 9. out_proj + residual
            r1 = pool_a.tile([128, 2, T], f32, tag="r1")
            for ob in range(2):
                po = psC.tile([128, T], f32, tag="mm")
                for db in range(2):
                    nc.tensor.matmul(po, r(w_outT[:, db, ob * 128:(ob + 1) * 128]),
                                     r(t1[:, db]), start=(db == 0), stop=(db == 1))
                nc.vector.tensor_add(r(r1[:, ob]), xf[:, ob], po)
            # 10. LN2
            h2 = layer_norm(r1, ln2w, ln2b, "2")
            # 11. fc1 + gelu (two halves of 4 f-blocks)
            ghalves = []
            for half in range(2):
                g = pool_g.tile([128, 4, T], f32, tag="g")
                for i in range(4):
                    fb = half * 4 + i
                    pf = psC.tile([128, T], f32, tag="mm")
                    for db in range(2):
                        nc.tensor.matmul(pf, r(w1T[:, db, fb * 128:(fb + 1) * 128]),
                                         r(h2[:, db]), start=(db == 0), stop=(db == 1))
                    act_gelu(r(g[:, i]), pf, bias=fc1b[:, fb:fb + 1])
                ghalves.append(g)
            # 12. fc2 + bias + residual -> out_fm
            ofm = pool_a.tile([128, 2, T], f32, tag="ofm")
            for ob in range(2):
                po = psC.tile([128, T], f32, tag="mm")
                for half in range(2):
                    for i in range(4):
                        fb = half * 4 + i
                        nc.tensor.matmul(
                            po, r(w2T[:, fb, ob * 128:(ob + 1) * 128]),
                            r(ghalves[half][:, i]),
                            start=(fb == 0), stop=(fb == 7))
                nc.vector.scalar_tensor_tensor(
                    out=ofm[:, ob], in0=po, scalar=fc2b[:, ob:ob + 1],
                    in1=r1[:, ob], op0=ALU.add, op1=ALU.add)
            # 13. transpose back to token-major and store
            o_tm = pool_a.tile([128, 4, 256], f32, tag="o_tm")
            for s in range(4):
                ps = psA.tile([128, 512], f32, tag="ptr")
                for ob in range(2):
                    nc.tensor.transpose(ps[:, ob * 128:(ob + 1) * 128],
                                        ofm[:, ob, s * 128:(s + 1) * 128], ident)
                nc.vector.tensor_copy(o_tm[:, s, :], ps[:, 0:256])
            nc.sync.dma_start(out=out_ap[tok, :].rearrange("(s p) d -> p s d", p=128),
                              in_=o_tm)
            prev_xc = xc
            prev_cum = cum

    nc.compile()
    return nc


def _get_nc(sim_compat=False):
    key = ("nc", sim_compat)
    if key not in _CACHE:
        _CACHE[key] = _build(sim_compat)
    return _CACHE[key]


_LAST_RESULTS = None


def kernel(**inputs) -> np.ndarray:
    global _LAST_RESULTS
    from concourse.bass_utils import run_bass_kernel_spmd

    nc = _get_nc()
    x = np.asarray(inputs["x"], np.float32)
    weights = {n: np.ascontiguousarray(np.asarray(inputs[n], np.float32))
               for n in WEIGHT_NAMES}
    in_maps = []
    for core in range(NCORES):
        m = {"x": np.ascontiguousarray(x[core])}
        m.update(weights)
        in_maps.append(m)
    res = run_bass_kernel_spmd(nc, in_maps, core_ids=list(range(NCORES)))
    _LAST_RESULTS = res
    return np.stack([r["out"] for r in res.results], axis=0)


if __name__ == "__main__":
    rng = np.random.default_rng(0)
    ins = {"x": rng.standard_normal((B, L, D), dtype=np.float32)}
    print("smoke build only")
    _get_nc()
    print("build OK")
